# revision 18
# baseline (speedup 1.0000x reference)
"""AttentionBlock kernel for 8 Trainium2 NeuronCores (Bass/Tile).

Problem (hardcoded shapes): x [16, 512, 32, 32] fp32, GroupNorm(32 groups,
eps=1e-5) -> 1x1-conv QKV (qkv_w [1536,512], qkv_b) -> 8-head attention over
T=1024 positions (head dim 64) -> 1x1-conv proj -> residual add.

Sharding: pure data-parallel over batch; each of the 8 cores handles 2
batches end-to-end; weights replicated; no collectives.

Per-core dataflow (per batch, all layouts channel-on-partition [128, ko, T]):
  1. GroupNorm stats per channel via bn_stats/bn_aggr (chunked x DMA so stats
     start while x streams in), group reduction via a tiny constant matmul,
     broadcast back via a second constant matmul, rstd via DVE-only
     Newton-rsqrt (keeps the ACT exp table resident - Sqrt lives in a
     different ACT table and a table swap costs ~1.5us), then tensor_scalar
     normalize.  norm_w/norm_b are folded into the QKV weights host-side, the
     attention scale and q bias are folded into Wq/bq, the k bias is dropped
     (softmax shift invariance), the v bias folded into the proj bias.
  2. q,k = Wqk @ h as [128, T] head-pairs; v^T computed as h^T @ Wv^T.
     St blocks for heads 0,1 are emitted in the middle of the QKV matmul
     stream (their inputs, the m=4 k-pair and m=0 q-pair chunks, are computed
     first) so ScalarE exp - the attention pacer - starts ~20us early.
  3. Per head: St = kz^T q in [s, t] layout (kz zero-padded to K=128 - PE
     tiling-mode switches corrupt in-flight matmuls on this HW, so every
     matmul stays in 128-row mode), exp on ScalarE (psum->sbuf, bf16),
     AV+denominator in one matmul with lhsT = [v^T | ones].  1/D via
     magic-seed + ONE Newton iteration computed directly from PSUM
     (z1 = (D*y0-2)*y0 = -1/D approx; the sign is fixed for free in the
     final fused multiply a = (av * -1) * z1).  One sbuf->sbuf DMA
     lane-shifts z1 onto the numerator partitions.  Software pipeline depth
     2: head h's St/exp stream interleaves with head h-2's AV matmuls.
  4. proj matmul + (residual + proj bias) add, out DMA chunked per m.

Cross-batch pipeline: batch 1's GroupNorm stats are emitted inside batch 0's
attention stream (DVE slack), and batch 1's QKV/vT matmuls sit between batch
0's attention and batch 0's proj in the PE program order, so the PE never
sits idle waiting for batch 0's last softmax-normalize chain and never
HAM-cools mid-kernel.
"""

import numpy as np

B, C, T = 16, 512, 1024
NH, CH = 8, 64
NG = 32
EPS = 1e-5
NCORES = 8
BPC = B // NCORES  # batches per core
KO = C // 128      # channel chunks

MM_QKV = 'bf16'
MM_ATT = 'bf16'
MM_PROJ = 'bf16'
TRACE = False


def _npdt(mode):
    import ml_dtypes
    return np.dtype(ml_dtypes.bfloat16) if mode == 'bf16' else np.float32


def _build_nc():
    import concourse.bass as bass
    import concourse.tile as tile
    from concourse import bacc, mybir
    from contextlib import ExitStack

    f32 = mybir.dt.float32
    f32r = mybir.dt.float32r
    bf16 = mybir.dt.bfloat16
    i32 = mybir.dt.int32

    def mmdt(mode):
        return {'bf16': bf16, 'f32r': f32r, 'f32': f32}[mode]

    dt_h = mmdt(MM_QKV)
    dt_att = mmdt(MM_ATT)
    dt_a = mmdt(MM_PROJ)

    nc = bacc.Bacc()
    AF = mybir.ActivationFunctionType
    ALU = mybir.AluOpType

    x_d = nc.dram_tensor("x", [BPC, 128, KO, T], f32, kind="ExternalInput")
    wqk_d = nc.dram_tensor("wqkT", [128, KO, 2 * C], mmdt(MM_QKV), kind="ExternalInput")
    wv_d = nc.dram_tensor("wvT", [128, KO, C], mmdt(MM_QKV), kind="ExternalInput")
    wp_d = nc.dram_tensor("wpT", [128, KO, C], mmdt(MM_PROJ), kind="ExternalInput")
    bq_d = nc.dram_tensor("bq", [128, KO], f32, kind="ExternalInput")
    bp_d = nc.dram_tensor("bp", [128, KO], f32, kind="ExternalInput")
    g_d = nc.dram_tensor("gmat", [128, KO, NG], f32, kind="ExternalInput")
    b_d = nc.dram_tensor("bmat", [128, KO, 128], f32, kind="ExternalInput")
    ones_d = nc.dram_tensor("ones", [128, 64], mmdt(MM_ATT), kind="ExternalInput")
    out_d = nc.dram_tensor("out", [BPC, 128, KO, T], f32, kind="ExternalOutput")

    # Every matmul keeps the PE in the default 128-row tiling mode (operands
    # zero-padded to K=128 where needed).  Switching the array tiling mode
    # without a drain corrupts in-flight matmuls on HW.
    def mm(out, lhsT, rhs, **kw):
        assert lhsT.partition_size() == 128
        return nc.tensor.matmul(out, lhsT, rhs, **kw)

    with tile.TileContext(nc) as tc, ExitStack() as ctx:
        consts = ctx.enter_context(tc.tile_pool(name="consts", bufs=1))
        xp = ctx.enter_context(tc.tile_pool(name="xp", bufs=2))
        hp = ctx.enter_context(tc.tile_pool(name="hp", bufs=1))
        qkp = ctx.enter_context(tc.tile_pool(name="qkp", bufs=2))
        esp = ctx.enter_context(tc.tile_pool(name="esp", bufs=24))
        rp = ctx.enter_context(tc.tile_pool(name="rp", bufs=2))
        ap_ = ctx.enter_context(tc.tile_pool(name="ap", bufs=2))
        gnp = ctx.enter_context(tc.tile_pool(name="gnp", bufs=2))
        psS = ctx.enter_context(tc.tile_pool(name="psS", bufs=2, space="PSUM"))
        psB = ctx.enter_context(tc.tile_pool(name="psB", bufs=2, space="PSUM"))

        # ---- batch-0 x DMA first (chunked per ko) so GN stats start early
        x_tiles = [None, None]
        x_tiles[0] = xp.tile([128, KO, T], f32, tag="x", name="x0")
        for ko in range(KO):
            for j in range(2):
                sl = slice(512 * j, 512 * (j + 1))
                nc.sync.dma_start(x_tiles[0][:, ko, sl], x_d[0, :, ko, sl])

        # ---- constants (after x chunk DMAs in queue order)
        g_sb = consts.tile([128, KO, NG], f32)
        nc.sync.dma_start(g_sb[:], g_d[:])
        bm_sb = consts.tile([128, KO, 128], f32)
        nc.sync.dma_start(bm_sb[:], b_d[:])
        wqk_sb = consts.tile([128, KO, 2 * C], mmdt(MM_QKV))
        nc.sync.dma_start(wqk_sb[:], wqk_d[:])
        wv_sb = consts.tile([128, KO, C], mmdt(MM_QKV))
        nc.sync.dma_start(wv_sb[:], wv_d[:])
        bq_sb = consts.tile([128, KO], f32)
        nc.sync.dma_start(bq_sb[:], bq_d[:])
        bp_sb = consts.tile([128, KO], f32)
        nc.sync.dma_start(bp_sb[:], bp_d[:])

        # prefetch batch-1 x before the proj weight (x1 gates batch-1 GN,
        # needed ~40us in; wp not until ~100us)
        x_tiles[1] = xp.tile([128, KO, T], f32, tag="x", name="x1")
        for ko in range(KO):
            nc.sync.dma_start(x_tiles[1][:, ko, :], x_d[1, :, ko, :])

        wp_sb = consts.tile([128, KO, C], mmdt(MM_PROJ))
        nc.sync.dma_start(wp_sb[:], wp_d[:])

        # v^T lhsT buffer: per head-pair p the 192 columns are
        # [vT_even(64) | ones(64) | vT_odd(64)]; head 2p uses cols 0:128 and
        # head 2p+1 uses cols 64:192.  The ones block is constant -> one DMA.
        vt_sb = consts.tile([128, 8, 4, 192], dt_att)
        ones_src = bass.AP(tensor=ones_d, offset=0,
                           ap=[[64, 128], [0, 32], [1, 64]])
        vt_flat = vt_sb[:].rearrange("p a b w -> p (a b) w")
        nc.sync.dma_start(vt_flat[:, :, 64:128], ones_src)

        # HAM warm-up scratch: zeroed bf16 tile for dummy matmuls
        warm_sb = consts.tile([128, 512], bf16)
        nc.gpsimd.memset(warm_sb[:], 0.0)
        warm_ps = psB.tile([128, 512], f32, tag="av", name="warm")
        for _ in range(44):
            nc.tensor.matmul(warm_ps[:], warm_sb[:, 0:128], warm_sb[:],
                             start=True, stop=True)

        # magic seed for Newton reciprocal (1/D): y0_bits = 0x7EF127EA - x_bits
        magic_sb = consts.tile([128, 2], i32)
        nc.vector.memset(magic_sb[:], 0x7EF127EA)
        # constant 2.0 broadcast tile for the GpSimd Newton step
        two_sb = consts.tile([128, 2], f32)
        nc.vector.memset(two_sb[:], 2.0)
        # magic seed for Newton rsqrt (GroupNorm): 0x5f3759df
        rsm_sb = consts.tile([NG, 1], i32)
        nc.vector.memset(rsm_sb[:], 0x5F3759DF)

        # kz zero-padding: head h's k occupies partitions 64*(h%2)..+64, the
        # other half stays zero forever -> memset once, outside the batch loop.
        kz_sb = consts.tile([128, NH, T], dt_att)
        nc.gpsimd.memset(kz_sb[64:128, 0:NH:2, :], 0.0)
        nc.gpsimd.memset(kz_sb[0:64, 1:NH:2, :], 0.0)

        # [mean | rstd] per group, zero-padded to 128 rows for the broadcast
        # matmul (rhs K must be 128); rows NG..127 stay zero forever.
        gst2 = consts.tile([128, 2], f32)
        nc.vector.memset(gst2[:], 0.0)

        # ---------------- stage helpers ----------------
        def gn_stats(b):
            """bn stats + group reduce + rstd -> writes gst2[0:NG, 0:2]."""
            x_sb = x_tiles[b]
            rhs3 = gnp.tile([128, KO, 3], f32, tag="rhs3")
            for ko in range(KO):
                stats = gnp.tile([128, 2, 6], f32, tag="stats")
                for j in range(2):
                    nc.vector.bn_stats(out=stats[:, j, :],
                                       in_=x_sb[:, ko, 512 * j:512 * (j + 1)])
                nc.vector.bn_aggr(out=rhs3[:, ko, 0:2], in_=stats[:])
                nc.vector.tensor_mul(rhs3[:, ko, 2:3], rhs3[:, ko, 0:1], rhs3[:, ko, 0:1])
            gps = psS.tile([NG, 3], f32, tag="st")
            for ko in range(KO):
                mm(gps[:], g_sb[:, ko, :], rhs3[:, ko, :],
                   start=(ko == 0), stop=(ko == KO - 1))
            gq = gnp.tile([NG, 3], f32, tag="gq")
            nc.vector.tensor_copy(gq[:], gps[:])
            gtmp = gnp.tile([NG, 4], f32, tag="gtmp")
            nc.vector.tensor_copy(gst2[0:NG, 0:1], gq[:, 0:1])
            # v = E[var] + E[mean^2] - mean^2 + eps
            nc.vector.tensor_add(gtmp[:, 0:1], gq[:, 1:2], gq[:, 2:3])
            nc.vector.tensor_mul(gtmp[:, 1:2], gq[:, 0:1], gq[:, 0:1])
            nc.vector.scalar_tensor_tensor(
                out=gtmp[:, 0:1], in0=gtmp[:, 0:1], scalar=EPS,
                in1=gtmp[:, 1:2], op0=ALU.add, op1=ALU.subtract)
            # rstd = 1/sqrt(v) via magic seed + 2 Newton iterations (DVE only;
            # keeps the ACT exp table resident).
            v = gtmp[:, 0:1]
            y = gtmp[:, 2:3]
            u = gtmp[:, 3:4]
            nc.vector.tensor_scalar(
                out=y.bitcast(i32), in0=v.bitcast(i32), scalar1=1,
                scalar2=None, op0=ALU.arith_shift_right)
            nc.vector.tensor_tensor(out=y.bitcast(i32), in0=rsm_sb[:],
                                    in1=y.bitcast(i32), op=ALU.subtract)
            for _ in range(2):
                nc.vector.tensor_mul(u, y, y)        # u = y^2
                nc.vector.tensor_mul(u, u, v)        # u = v*y^2
                nc.vector.scalar_tensor_tensor(      # y = (u-3)*y = -2*y'
                    out=y, in0=u, scalar=3.0, in1=y,
                    op0=ALU.subtract, op1=ALU.mult)
                nc.vector.tensor_scalar_mul(y, y, -0.5)
            nc.vector.tensor_copy(gst2[0:NG, 1:2], y)

        def gn_bcast(b):
            """Broadcast [mean|rstd] to channels -> bst [128, 2*KO] sbuf."""
            bst_ps = psS.tile([128, 2 * KO], f32, tag="st")
            for ko in range(KO):
                mm(bst_ps[:, 2 * ko:2 * ko + 2], bm_sb[:, ko, :], gst2[:],
                   start=True, stop=True)
            bst = gnp.tile([128, 2 * KO], f32, tag="bst_sb")
            nc.vector.tensor_copy(bst[:], bst_ps[:])
            return bst

        def gn_normalize(b, bst, h_sb):
            """h = (x - mean) * rstd."""
            x_sb = x_tiles[b]
            for ko in range(KO):
                nc.vector.tensor_scalar(
                    out=h_sb[:, ko, :], in0=x_sb[:, ko, :],
                    scalar1=bst[:, 2 * ko:2 * ko + 1],
                    scalar2=bst[:, 2 * ko + 1:2 * ko + 2],
                    op0=ALU.subtract, op1=ALU.mult)

        def bp_add(b):
            """x += bp (residual bias) - only needed before proj's residual
            add, so emitted late to keep it off the QKV critical path."""
            x_sb = x_tiles[b]
            for ko in range(KO):
                nc.vector.tensor_scalar(
                    out=x_sb[:, ko, :], in0=x_sb[:, ko, :],
                    scalar1=bp_sb[:, ko:ko + 1], scalar2=None, op0=ALU.add)

        def qkv_group(m, h_sb, q_sb):
            """One QKV output chunk m, full T width (N=1024 matmuls)."""
            pq = psS.tile([128, T], f32, tag="st")
            for half in range(2):
                sl = slice(512 * half, 512 * (half + 1))
                for ko in range(KO):
                    mm(pq[:, sl], wqk_sb[:, ko, 128 * m:128 * (m + 1)],
                       h_sb[:, ko, sl], start=(ko == 0), stop=(ko == KO - 1))
            if m < 4:
                nc.vector.tensor_scalar(
                    out=q_sb[:, m, :], in0=pq[:],
                    scalar1=bq_sb[:, m:m + 1], scalar2=None, op0=ALU.add)
            else:
                p = m - 4
                nc.vector.tensor_copy(kz_sb[0:64, 2 * p, :], pq[0:64, :])
                nc.vector.tensor_copy(kz_sb[64:128, 2 * p + 1, :], pq[64:128, :])

        def vt_group(tc_i, h_sb):
            pv = psS.tile([128, 512], f32, tag="st")
            for ko in range(KO):
                mm(pv[:], h_sb[:, ko, 128 * tc_i:128 * (tc_i + 1)],
                   wv_sb[:, ko, :], start=(ko == 0), stop=(ko == KO - 1))
            pvv = pv[:].rearrange("p (h c) -> p h c", c=CH)
            nc.vector.tensor_copy(vt_sb[:, tc_i, :, 0:64], pvv[:, 0:NH:2, :])
            nc.vector.tensor_copy(vt_sb[:, tc_i, :, 128:192], pvv[:, 1:NH:2, :])

        def st_block(h, q_sb, av_hook=None):
            """St + exp for head h -> 8 es tiles. av_hook(sc) interleaves the
            previous head's AV matmuls into the St stream."""
            p = h // 2
            es_tiles = []
            for sc in range(8):
                es = esp.tile([128, T], dt_att, tag="es")
                st = psS.tile([128, T], f32, tag="st")
                for half in range(2):
                    sl = slice(512 * half, 512 * (half + 1))
                    mm(st[:, sl], kz_sb[:, h, 128 * sc:128 * (sc + 1)],
                       q_sb[:, p, sl], start=True, stop=True)
                nc.scalar.activation(es[:], st[:], AF.Exp)
                if av_hook is not None:
                    av_hook(sc)
                es_tiles.append(es)
            return es_tiles

        def av_mms(avp, h_av, es_av, sc):
            p, e = h_av // 2, h_av % 2
            for half in range(2):
                sl = slice(512 * half, 512 * (half + 1))
                mm(avp[:, sl], vt_sb[:, sc, p, 64 * e:64 * e + 128],
                   es_av[sc][:, sl], start=(sc == 0), stop=(sc == 7))

        def finish_norm(h_av, av, a_sb, fast=False):
            """a = av / D via 1-iteration Newton from the magic seed.
            z1 = (D*y0 - 2)*y0 = -(1/D approx); the final multiply computes
            a = (av * -1) * z1 so no separate sign fix is needed.  The
            (t-2)*y0 step runs on GpSimd (two plain tensor_tensor ops - the
            fused stt opcode is illegal on Pool); fast=True keeps it on DVE
            as one stt for the batch-final head whose chain gates proj."""
            p, e = h_av // 2, h_av % 2
            b0, b1 = 64 * e, 64 * (1 - e)
            y0 = rp.tile([128, T], f32, tag="y0")
            z = rp.tile([128, T], f32, tag="z")
            if fast:
                # per-half pipelined all-DVE chain: half 0's lane-shift DMA
                # overlaps half 1's compute (used for the batch-final heads
                # whose chain latency gates proj)
                for sl in (slice(0, 512), slice(512, T)):
                    nc.vector.tensor_tensor(
                        out=y0[b1:b1 + 64, sl].bitcast(i32),
                        in0=magic_sb[b1:b1 + 64, 0:1].to_broadcast((64, 512)),
                        in1=av[b1:b1 + 64, sl].bitcast(i32), op=ALU.subtract)
                    nc.vector.tensor_tensor(
                        out=z[b1:b1 + 64, sl], in0=av[b1:b1 + 64, sl],
                        in1=y0[b1:b1 + 64, sl], op=ALU.mult)
                    nc.vector.scalar_tensor_tensor(
                        out=z[b1:b1 + 64, sl], in0=z[b1:b1 + 64, sl],
                        scalar=2.0, in1=y0[b1:b1 + 64, sl],
                        op0=ALU.subtract, op1=ALU.mult)
                    nc.sync.dma_start(out=z[b0:b0 + 64, sl], in_=z[b1:b1 + 64, sl])
                    nc.vector.scalar_tensor_tensor(
                        out=a_sb[b0:b0 + 64, p, sl],
                        in0=av[b0:b0 + 64, sl], scalar=-1.0,
                        in1=z[b0:b0 + 64, sl], op0=ALU.mult, op1=ALU.mult)
                return
            nc.vector.tensor_tensor(   # y0 = bits(magic - D_bits)
                out=y0[b1:b1 + 64, :].bitcast(i32),
                in0=magic_sb[b1:b1 + 64, 0:1].to_broadcast((64, T)),
                in1=av[b1:b1 + 64, :].bitcast(i32), op=ALU.subtract)
            nc.vector.tensor_tensor(   # z = D*y0
                out=z[b1:b1 + 64, :], in0=av[b1:b1 + 64, :],
                in1=y0[b1:b1 + 64, :], op=ALU.mult)
            nc.gpsimd.tensor_tensor(
                out=z[b1:b1 + 64, :], in0=z[b1:b1 + 64, :],
                in1=two_sb[b1:b1 + 64, 0:1].to_broadcast((64, T)),
                op=ALU.subtract)
            nc.gpsimd.tensor_tensor(
                out=z[b1:b1 + 64, :], in0=z[b1:b1 + 64, :],
                in1=y0[b1:b1 + 64, :], op=ALU.mult)
            nc.sync.dma_start(out=z[b0:b0 + 64, :], in_=z[b1:b1 + 64, :])
            nc.vector.scalar_tensor_tensor(
                out=a_sb[b0:b0 + 64, p, :],
                in0=av[b0:b0 + 64, :], scalar=-1.0,
                in1=z[b0:b0 + 64, :], op0=ALU.mult, op1=ALU.mult)

        def proj_out(b, a_sb, ms=None):
            x_sb = x_tiles[b]
            for m in (range(KO) if ms is None else ms):
                po = psS.tile([128, T], f32, tag="st")
                for half in range(2):
                    sl = slice(512 * half, 512 * (half + 1))
                    for ko in (1, 2, 3, 0):
                        mm(po[:, sl], wp_sb[:, ko, 128 * m:128 * (m + 1)],
                           a_sb[:, ko, sl], start=(ko == 1), stop=(ko == 0))
                nc.vector.tensor_add(x_sb[:, m, :], po[:], x_sb[:, m, :])
                nc.sync.dma_start(out_d[b, :, m, :], x_sb[:, m, :])

        def proj_out_waves(b, a_sb):
            """Final-batch proj: the ko1-3 accumulations for ALL m run first
            (they only need heads 2..7, long done) so the PE stays warm while
            the tail pair's softmax-normalize chains finish; only the ko0
            wave waits on them.  Uses psB for 2 extra open PSUM groups."""
            x_sb = x_tiles[b]
            pos = []
            for m in range(KO):
                pool = psS if m < 2 else psB
                pos.append(pool.tile([128, T], f32, tag="st" if m < 2 else "av",
                                     name=f"po{m}"))
            for ko in (1, 2, 3):
                for m in range(KO):
                    for half in range(2):
                        sl = slice(512 * half, 512 * (half + 1))
                        mm(pos[m][:, sl], wp_sb[:, ko, 128 * m:128 * (m + 1)],
                           a_sb[:, ko, sl], start=(ko == 1), stop=False)
            for m in range(KO):
                for half in range(2):
                    sl = slice(512 * half, 512 * (half + 1))
                    mm(pos[m][:, sl], wp_sb[:, 0, 128 * m:128 * (m + 1)],
                       a_sb[:, 0, sl], start=False, stop=True)
                nc.vector.tensor_add(x_sb[:, m, :], pos[m][:], x_sb[:, m, :])
                nc.sync.dma_start(out_d[b, :, m, :], x_sb[:, m, :])

        # ---------------- emission ----------------
        # Stage A(0): GroupNorm batch 0
        gn_stats(0)
        h_sbs = [None, None]
        q_sbs = [None, None]
        a_sbs = [None, None]
        h_sbs[0] = hp.tile([128, KO, T], dt_h, tag="h", name="h0")
        gn_normalize(0, gn_bcast(0), h_sbs[0])

        FIRST_MS = (5, 1)            # k/q chunks of head pair 1
        REST_MS = (6, 2, 7, 3, 4, 0)  # pairs 2,3 then pair 0
        HEAD_LOOP = (4, 5, 6, 7, 0, 1)

        def stage_B(b):
            """QKV + vT with St(h0)/St(h1) spliced in right after their
            inputs (k pair 0 = m4, q pair 0 = m0) are available, so ScalarE
            exp starts ~20us before the attention stage.  The vT groups are
            interleaved into St(h0)'s stream and the remaining QKV groups
            into St(h1)'s, so the PE stays dense while exp paces the St
            psum ring."""
            h_sb, q_sb = h_sbs[b], q_sbs[b]
            es01 = {}
            for m in FIRST_MS:
                qkv_group(m, h_sb, q_sb)
            es01[0] = st_block(2, q_sb,
                               av_hook=lambda sc: vt_group(sc, h_sb))

            def qkv_hook(sc):
                if sc < len(REST_MS):
                    qkv_group(REST_MS[sc], h_sb, q_sb)
            es01[1] = st_block(3, q_sb, av_hook=qkv_hook)
            return es01

        def stage_C(b, es01, hooks=None):
            """Attention heads: St(h) interleaved with AV(h-2); finishes."""
            q_sb, a_sb = q_sbs[b], a_sbs[b]
            es_prev2, es_prev1 = es01[0], es01[1]  # pending-AV heads
            hp2, hp1 = 2, 3
            for i, h in enumerate(HEAD_LOOP):
                avp = psB.tile([128, T], f32, tag="av", name="avp")
                hook = (lambda sc: av_mms(avp, hp2, es_prev2, sc))
                es_h = st_block(h, q_sb, av_hook=hook)
                finish_norm(hp2, avp, a_sb)
                es_prev2, hp2 = es_prev1, hp1
                es_prev1, hp1 = es_h, h
                if hooks and i in hooks:
                    hooks[i]()
            # batch-final two heads (pair 0): first chain on GpSimd, second
            # all-DVE, so the two run concurrently and proj's last-accumulated
            # chunk (ko0) unblocks sooner.
            for es_t, h_t in ((es_prev2, hp2), (es_prev1, hp1)):
                avp = psB.tile([128, T], f32, tag="av", name="avp")
                for sc in range(8):
                    av_mms(avp, h_t, es_t, sc)
                finish_norm(h_t, avp, a_sb, fast=True)

        q_sbs[0] = qkp.tile([128, KO, T], dt_att, tag="q", name="q0")
        a_sbs[0] = ap_.tile([128, KO, T], dt_a, tag="a", name="a0")
        es01_0 = stage_B(0)

        # batch-1 GN lands in batch-0's attention DVE slack
        def hook_gn1():
            gn_stats(1)

        def hook_norm1():
            h_sbs[1] = hp.tile([128, KO, T], dt_h, tag="h", name="h1")
            gn_normalize(1, gn_bcast(1), h_sbs[1])

        bp_add(0)
        q_sbs[1] = qkp.tile([128, KO, T], dt_att, tag="q", name="q1")
        a_sbs[1] = ap_.tile([128, KO, T], dt_a, tag="a", name="a1")

        def stage_B1_main():
            h_sb, q_sb = h_sbs[1], q_sbs[1]
            es01 = {}
            es01[0] = st_block(2, q_sb,
                               av_hook=lambda sc: vt_group(sc, h_sb))

            def qkv_hook(sc):
                if sc < len(REST_MS):
                    qkv_group(REST_MS[sc], h_sb, q_sb)
            es01[1] = st_block(3, q_sb, av_hook=qkv_hook)
            return es01

        def hook_b1pre():
            for m in FIRST_MS:
                qkv_group(m, h_sbs[1], q_sbs[1])

        stage_C(0, es01_0, hooks={1: hook_gn1, 3: hook_norm1, 4: hook_b1pre})

        # batch-1 QKV/vT fill the PE while batch-0's last softmax-normalize
        # chain completes; then batch-0 proj.
        es01_1 = stage_B1_main()
        bp_add(1)
        stage_C(1, es01_1, hooks={
            1: lambda: proj_out(0, a_sbs[0], ms=[0, 1]),
            3: lambda: proj_out(0, a_sbs[0], ms=[2, 3]),
        })
        proj_out_waves(1, a_sbs[1])

    if not nc.is_finalized():
        nc.finalize()
    return nc


def _prep_inputs(x, norm_w, norm_b, qkv_w, qkv_b, proj_w, proj_b):
    """Fold norms/biases/scale into weights; reshape for the kernel layout."""
    f = np.float32
    x = np.asarray(x, f)
    nw = np.asarray(norm_w, f)
    nb = np.asarray(norm_b, f)
    qkv_w = np.asarray(qkv_w, f)
    qkv_b = np.asarray(qkv_b, f)
    proj_w = np.asarray(proj_w, f)
    proj_b = np.asarray(proj_b, f)

    Wq, Wk, Wv = qkv_w[0:C], qkv_w[C:2 * C], qkv_w[2 * C:3 * C]
    bqv, bkv, bvv = qkv_b[0:C], qkv_b[C:2 * C], qkv_b[2 * C:3 * C]
    scale = f(1.0 / np.sqrt(CH))
    Wq_e = (Wq * nw[None, :]) * scale
    bq_e = (Wq @ nb + bqv) * scale
    Wk_e = Wk * nw[None, :]          # k bias dropped (softmax shift invariance)
    Wv_e = Wv * nw[None, :]
    bv_e = Wv @ nb + bvv
    bp_e = proj_b + proj_w @ bv_e    # v bias folded into proj bias

    def chan_chunks(vec):  # [C] -> [128, KO]
        return np.ascontiguousarray(vec.reshape(KO, 128).T)

    def lhsT_chunks(wT, dtype):  # [C, M] -> [128, KO, M]
        return np.ascontiguousarray(
            wT.reshape(KO, 128, wT.shape[1]).transpose(1, 0, 2)).astype(dtype)

    wqkT = np.concatenate([Wq_e, Wk_e], axis=0).T  # [C, 1024]
    gm = np.zeros((C, NG), f)
    gm[np.arange(C), np.arange(C) // (C // NG)] = 1.0 / (C // NG)
    bm = np.zeros((128, C), f)
    bm[np.arange(C) // (C // NG), np.arange(C)] = 1.0

    dqkv = _npdt(MM_QKV)
    dproj = _npdt(MM_PROJ)
    shared = {
        "wqkT": lhsT_chunks(wqkT, dqkv),
        "wvT": lhsT_chunks(Wv_e.T, dqkv),
        "wpT": lhsT_chunks(proj_w.T, dproj),
        "bq": chan_chunks(bq_e),
        "bp": chan_chunks(bp_e),
        "gmat": np.ascontiguousarray(
            gm.reshape(KO, 128, NG).transpose(1, 0, 2)),
        "bmat": np.ascontiguousarray(bm.reshape(128, KO, 128)),
        "ones": np.ones((128, 64), _npdt(MM_ATT)),
    }
    xr = x.reshape(B, C, T)
    in_maps = []
    for c in range(NCORES):
        xc = xr[c * BPC:(c + 1) * BPC].reshape(BPC, KO, 128, T).transpose(0, 2, 1, 3)
        m = dict(shared)
        m["x"] = np.ascontiguousarray(xc)
        in_maps.append(m)
    return in_maps


def kernel(x, norm_w, norm_b, qkv_w, qkv_b, proj_w, proj_b):
    from concourse.bass_utils import run_bass_kernel_spmd

    in_maps = _prep_inputs(x, norm_w, norm_b, qkv_w, qkv_b, proj_w, proj_b)
    nc = _build_nc()
    res = run_bass_kernel_spmd(nc, in_maps, core_ids=list(range(NCORES)), trace=TRACE)
    kernel.last_results = res
    outs = []
    for c in range(NCORES):
        oc = res.results[c]["out"]  # [BPC, 128, KO, T]
        outs.append(np.asarray(oc).transpose(0, 2, 1, 3).reshape(BPC, C, T))
    full = np.concatenate(outs, axis=0).reshape(B, C, 32, 32).astype(np.float32)
    return full


# revision 19
# speedup vs baseline: 1.0600x; 1.0600x over previous
"""AttentionBlock kernel for 8 Trainium2 NeuronCores (Bass/Tile).

Problem (hardcoded shapes): x [16, 512, 32, 32] fp32, GroupNorm(32 groups,
eps=1e-5) -> 1x1-conv QKV (qkv_w [1536,512], qkv_b) -> 8-head attention over
T=1024 positions (head dim 64) -> 1x1-conv proj -> residual add.

Sharding: pure data-parallel over batch; each of the 8 cores handles 2
batches end-to-end; weights replicated; no collectives.

Per-core dataflow (per batch, all layouts channel-on-partition [128, ko, T]):
  1. GroupNorm stats per channel via bn_stats/bn_aggr (chunked x DMA so stats
     start while x streams in), group reduction via a tiny constant matmul,
     broadcast back via a second constant matmul, rstd via DVE-only
     Newton-rsqrt (keeps the ACT exp table resident - Sqrt lives in a
     different ACT table and a table swap costs ~1.5us), then tensor_scalar
     normalize.  norm_w/norm_b are folded into the QKV weights host-side, the
     attention scale and q bias are folded into Wq/bq, the k bias is dropped
     (softmax shift invariance), the v bias folded into the proj bias.
  2. q,k = Wqk @ h as [128, T] head-pairs; v^T computed as h^T @ Wv^T.
     St blocks for heads 0,1 are emitted in the middle of the QKV matmul
     stream (their inputs, the m=4 k-pair and m=0 q-pair chunks, are computed
     first) so ScalarE exp - the attention pacer - starts ~20us early.
  3. Per head: St = kz^T q in [s, t] layout (kz zero-padded to K=128 - PE
     tiling-mode switches corrupt in-flight matmuls on this HW, so every
     matmul stays in 128-row mode), exp on ScalarE (psum->sbuf, bf16),
     AV+denominator in one matmul with lhsT = [v^T | ones].  1/D via
     magic-seed + ONE Newton iteration computed directly from PSUM
     (z1 = (D*y0-2)*y0 = -1/D approx; the sign is fixed for free in the
     final fused multiply a = (av * -1) * z1).  One sbuf->sbuf DMA
     lane-shifts z1 onto the numerator partitions.  Software pipeline depth
     2: head h's St/exp stream interleaves with head h-2's AV matmuls.
  4. proj matmul + (residual + proj bias) add, out DMA chunked per m.

Cross-batch pipeline: batch 1's GroupNorm stats are emitted inside batch 0's
attention stream (DVE slack), and batch 1's QKV/vT matmuls sit between batch
0's attention and batch 0's proj in the PE program order, so the PE never
sits idle waiting for batch 0's last softmax-normalize chain and never
HAM-cools mid-kernel.
"""

import numpy as np

B, C, T = 16, 512, 1024
NH, CH = 8, 64
NG = 32
EPS = 1e-5
NCORES = 8
BPC = B // NCORES  # batches per core
KO = C // 128      # channel chunks

MM_QKV = 'bf16'
MM_ATT = 'bf16'
MM_PROJ = 'bf16'
TRACE = False


def _npdt(mode):
    import ml_dtypes
    return np.dtype(ml_dtypes.bfloat16) if mode == 'bf16' else np.float32


def _build_nc():
    import concourse.bass as bass
    import concourse.tile as tile
    from concourse import bacc, mybir
    from contextlib import ExitStack

    f32 = mybir.dt.float32
    f32r = mybir.dt.float32r
    bf16 = mybir.dt.bfloat16
    i32 = mybir.dt.int32

    def mmdt(mode):
        return {'bf16': bf16, 'f32r': f32r, 'f32': f32}[mode]

    dt_h = mmdt(MM_QKV)
    dt_att = mmdt(MM_ATT)
    dt_a = mmdt(MM_PROJ)

    nc = bacc.Bacc()
    AF = mybir.ActivationFunctionType
    ALU = mybir.AluOpType

    x_d = nc.dram_tensor("x", [BPC, 128, KO, T], f32, kind="ExternalInput")
    wqk_d = nc.dram_tensor("wqkT", [128, KO, 2 * C], mmdt(MM_QKV), kind="ExternalInput")
    wv_d = nc.dram_tensor("wvT", [128, KO, C], mmdt(MM_QKV), kind="ExternalInput")
    wp_d = nc.dram_tensor("wpT", [128, KO, C], mmdt(MM_PROJ), kind="ExternalInput")
    bq_d = nc.dram_tensor("bq", [128, KO], f32, kind="ExternalInput")
    bp_d = nc.dram_tensor("bp", [128, KO], f32, kind="ExternalInput")
    g_d = nc.dram_tensor("gmat", [128, KO, NG], f32, kind="ExternalInput")
    b_d = nc.dram_tensor("bmat", [128, KO, 128], f32, kind="ExternalInput")
    ones_d = nc.dram_tensor("ones", [128, 64], mmdt(MM_ATT), kind="ExternalInput")
    out_d = nc.dram_tensor("out", [BPC, 128, KO, T], f32, kind="ExternalOutput")

    # Every matmul keeps the PE in the default 128-row tiling mode (operands
    # zero-padded to K=128 where needed).  Switching the array tiling mode
    # without a drain corrupts in-flight matmuls on HW.
    def mm(out, lhsT, rhs, **kw):
        assert lhsT.partition_size() == 128
        return nc.tensor.matmul(out, lhsT, rhs, **kw)

    with tile.TileContext(nc) as tc, ExitStack() as ctx:
        consts = ctx.enter_context(tc.tile_pool(name="consts", bufs=1))
        xp = ctx.enter_context(tc.tile_pool(name="xp", bufs=2))
        hp = ctx.enter_context(tc.tile_pool(name="hp", bufs=1))
        qkp = ctx.enter_context(tc.tile_pool(name="qkp", bufs=2))
        esp = ctx.enter_context(tc.tile_pool(name="esp", bufs=24))
        rp = ctx.enter_context(tc.tile_pool(name="rp", bufs=2))
        ap_ = ctx.enter_context(tc.tile_pool(name="ap", bufs=2))
        gnp = ctx.enter_context(tc.tile_pool(name="gnp", bufs=2))
        psS = ctx.enter_context(tc.tile_pool(name="psS", bufs=2, space="PSUM"))
        psB = ctx.enter_context(tc.tile_pool(name="psB", bufs=2, space="PSUM"))

        # ---- batch-0 x DMA first (chunked per ko) so GN stats start early
        x_tiles = [None, None]
        x_tiles[0] = xp.tile([128, KO, T], f32, tag="x", name="x0")
        for ko in range(KO):
            for j in range(2):
                sl = slice(512 * j, 512 * (j + 1))
                nc.sync.dma_start(x_tiles[0][:, ko, sl], x_d[0, :, ko, sl])

        # ---- constants (after x chunk DMAs in queue order)
        g_sb = consts.tile([128, KO, NG], f32)
        nc.sync.dma_start(g_sb[:], g_d[:])
        bm_sb = consts.tile([128, KO, 128], f32)
        nc.sync.dma_start(bm_sb[:], b_d[:])
        wqk_sb = consts.tile([128, KO, 2 * C], mmdt(MM_QKV))
        nc.sync.dma_start(wqk_sb[:], wqk_d[:])
        wv_sb = consts.tile([128, KO, C], mmdt(MM_QKV))
        nc.sync.dma_start(wv_sb[:], wv_d[:])
        bq_sb = consts.tile([128, KO], f32)
        nc.sync.dma_start(bq_sb[:], bq_d[:])
        bp_sb = consts.tile([128, KO], f32)
        nc.sync.dma_start(bp_sb[:], bp_d[:])

        # prefetch batch-1 x before the proj weight (x1 gates batch-1 GN,
        # needed ~40us in; wp not until ~100us)
        x_tiles[1] = xp.tile([128, KO, T], f32, tag="x", name="x1")
        for ko in range(KO):
            nc.sync.dma_start(x_tiles[1][:, ko, :], x_d[1, :, ko, :])

        wp_sb = consts.tile([128, KO, C], mmdt(MM_PROJ))
        nc.sync.dma_start(wp_sb[:], wp_d[:])

        # v^T lhsT buffer: per head-pair p the 192 columns are
        # [vT_even(64) | ones(64) | vT_odd(64)]; head 2p uses cols 0:128 and
        # head 2p+1 uses cols 64:192.  The ones block is constant -> one DMA.
        vt_sb = consts.tile([128, 8, 4, 192], dt_att)
        ones_src = bass.AP(tensor=ones_d, offset=0,
                           ap=[[64, 128], [0, 32], [1, 64]])
        vt_flat = vt_sb[:].rearrange("p a b w -> p (a b) w")
        nc.sync.dma_start(vt_flat[:, :, 64:128], ones_src)

        # HAM warm-up scratch: zeroed bf16 tile for dummy matmuls
        warm_sb = consts.tile([128, 512], bf16)
        nc.gpsimd.memset(warm_sb[:], 0.0)
        warm_ps = psB.tile([128, 512], f32, tag="av", name="warm")
        for _ in range(32):
            nc.tensor.matmul(warm_ps[:], warm_sb[:, 0:128], warm_sb[:],
                             start=True, stop=True)

        # magic seed for Newton reciprocal (1/D): y0_bits = 0x7EF127EA - x_bits
        magic_sb = consts.tile([128, 2], i32)
        nc.vector.memset(magic_sb[:], 0x7EF127EA)
        # constant 2.0 broadcast tile for the GpSimd Newton step
        two_sb = consts.tile([128, 2], f32)
        nc.vector.memset(two_sb[:], 2.0)
        # magic seed for Newton rsqrt (GroupNorm): 0x5f3759df
        rsm_sb = consts.tile([NG, 1], i32)
        nc.vector.memset(rsm_sb[:], 0x5F3759DF)

        # kz zero-padding: head h's k occupies partitions 64*(h%2)..+64, the
        # other half stays zero forever -> memset once, outside the batch loop.
        kz_sb = consts.tile([128, NH, T], dt_att)
        nc.gpsimd.memset(kz_sb[64:128, 0:NH:2, :], 0.0)
        nc.gpsimd.memset(kz_sb[0:64, 1:NH:2, :], 0.0)

        # [mean | rstd] per group, zero-padded to 128 rows for the broadcast
        # matmul (rhs K must be 128); rows NG..127 stay zero forever.
        gst2 = consts.tile([128, 2], f32)
        nc.vector.memset(gst2[:], 0.0)

        # ---------------- stage helpers ----------------
        def gn_stats(b):
            """bn stats + group reduce + rstd -> writes gst2[0:NG, 0:2]."""
            x_sb = x_tiles[b]
            rhs3 = gnp.tile([128, KO, 3], f32, tag="rhs3")
            for ko in range(KO):
                stats = gnp.tile([128, 2, 6], f32, tag="stats")
                for j in range(2):
                    nc.vector.bn_stats(out=stats[:, j, :],
                                       in_=x_sb[:, ko, 512 * j:512 * (j + 1)])
                nc.vector.bn_aggr(out=rhs3[:, ko, 0:2], in_=stats[:])
                nc.vector.tensor_mul(rhs3[:, ko, 2:3], rhs3[:, ko, 0:1], rhs3[:, ko, 0:1])
            gps = psS.tile([NG, 3], f32, tag="st")
            for ko in range(KO):
                mm(gps[:], g_sb[:, ko, :], rhs3[:, ko, :],
                   start=(ko == 0), stop=(ko == KO - 1))
            gq = gnp.tile([NG, 3], f32, tag="gq")
            nc.vector.tensor_copy(gq[:], gps[:])
            gtmp = gnp.tile([NG, 4], f32, tag="gtmp")
            nc.vector.tensor_copy(gst2[0:NG, 0:1], gq[:, 0:1])
            # v = E[var] + E[mean^2] - mean^2 + eps
            nc.vector.tensor_add(gtmp[:, 0:1], gq[:, 1:2], gq[:, 2:3])
            nc.vector.tensor_mul(gtmp[:, 1:2], gq[:, 0:1], gq[:, 0:1])
            nc.vector.scalar_tensor_tensor(
                out=gtmp[:, 0:1], in0=gtmp[:, 0:1], scalar=EPS,
                in1=gtmp[:, 1:2], op0=ALU.add, op1=ALU.subtract)
            # rstd = 1/sqrt(v) via magic seed + 2 Newton iterations (DVE only;
            # keeps the ACT exp table resident).
            v = gtmp[:, 0:1]
            y = gtmp[:, 2:3]
            u = gtmp[:, 3:4]
            nc.vector.tensor_scalar(
                out=y.bitcast(i32), in0=v.bitcast(i32), scalar1=1,
                scalar2=None, op0=ALU.arith_shift_right)
            nc.vector.tensor_tensor(out=y.bitcast(i32), in0=rsm_sb[:],
                                    in1=y.bitcast(i32), op=ALU.subtract)
            for _ in range(2):
                nc.vector.tensor_mul(u, y, y)        # u = y^2
                nc.vector.tensor_mul(u, u, v)        # u = v*y^2
                nc.vector.scalar_tensor_tensor(      # y = (u-3)*y = -2*y'
                    out=y, in0=u, scalar=3.0, in1=y,
                    op0=ALU.subtract, op1=ALU.mult)
                nc.vector.tensor_scalar_mul(y, y, -0.5)
            nc.vector.tensor_copy(gst2[0:NG, 1:2], y)

        def gn_bcast(b):
            """Broadcast [mean|rstd] to channels -> bst [128, 2*KO] sbuf."""
            bst_ps = psS.tile([128, 2 * KO], f32, tag="st")
            for ko in range(KO):
                mm(bst_ps[:, 2 * ko:2 * ko + 2], bm_sb[:, ko, :], gst2[:],
                   start=True, stop=True)
            bst = gnp.tile([128, 2 * KO], f32, tag="bst_sb")
            nc.vector.tensor_copy(bst[:], bst_ps[:])
            return bst

        def gn_normalize(b, bst, h_sb):
            """h = (x - mean) * rstd."""
            x_sb = x_tiles[b]
            for ko in range(KO):
                nc.vector.tensor_scalar(
                    out=h_sb[:, ko, :], in0=x_sb[:, ko, :],
                    scalar1=bst[:, 2 * ko:2 * ko + 1],
                    scalar2=bst[:, 2 * ko + 1:2 * ko + 2],
                    op0=ALU.subtract, op1=ALU.mult)

        def bp_add(b):
            """x += bp (residual bias) - only needed before proj's residual
            add, so emitted late to keep it off the QKV critical path."""
            x_sb = x_tiles[b]
            for ko in range(KO):
                nc.vector.tensor_scalar(
                    out=x_sb[:, ko, :], in0=x_sb[:, ko, :],
                    scalar1=bp_sb[:, ko:ko + 1], scalar2=None, op0=ALU.add)

        def qkv_group(m, h_sb, q_sb):
            """One QKV output chunk m, full T width (N=1024 matmuls)."""
            pq = psS.tile([128, T], f32, tag="st")
            for half in range(2):
                sl = slice(512 * half, 512 * (half + 1))
                for ko in range(KO):
                    mm(pq[:, sl], wqk_sb[:, ko, 128 * m:128 * (m + 1)],
                       h_sb[:, ko, sl], start=(ko == 0), stop=(ko == KO - 1))
            if m < 4:
                nc.vector.tensor_scalar(
                    out=q_sb[:, m, :], in0=pq[:],
                    scalar1=bq_sb[:, m:m + 1], scalar2=None, op0=ALU.add)
            else:
                p = m - 4
                nc.vector.tensor_copy(kz_sb[0:64, 2 * p, :], pq[0:64, :])
                nc.vector.tensor_copy(kz_sb[64:128, 2 * p + 1, :], pq[64:128, :])

        def vt_group(tc_i, h_sb):
            pv = psS.tile([128, 512], f32, tag="st")
            for ko in range(KO):
                mm(pv[:], h_sb[:, ko, 128 * tc_i:128 * (tc_i + 1)],
                   wv_sb[:, ko, :], start=(ko == 0), stop=(ko == KO - 1))
            pvv = pv[:].rearrange("p (h c) -> p h c", c=CH)
            nc.vector.tensor_copy(vt_sb[:, tc_i, :, 0:64], pvv[:, 0:NH:2, :])
            nc.vector.tensor_copy(vt_sb[:, tc_i, :, 128:192], pvv[:, 1:NH:2, :])

        def st_block(h, q_sb, av_hook=None):
            """St + exp for head h -> 8 es tiles. av_hook(sc) interleaves the
            previous head's AV matmuls into the St stream."""
            p = h // 2
            es_tiles = []
            for sc in range(8):
                es = esp.tile([128, T], dt_att, tag="es")
                st = psS.tile([128, T], f32, tag="st")
                for half in range(2):
                    sl = slice(512 * half, 512 * (half + 1))
                    mm(st[:, sl], kz_sb[:, h, 128 * sc:128 * (sc + 1)],
                       q_sb[:, p, sl], start=True, stop=True)
                nc.scalar.activation(es[:], st[:], AF.Exp)
                if av_hook is not None:
                    av_hook(sc)
                es_tiles.append(es)
            return es_tiles

        def av_mms(avp, h_av, es_av, sc):
            p, e = h_av // 2, h_av % 2
            for half in range(2):
                sl = slice(512 * half, 512 * (half + 1))
                mm(avp[:, sl], vt_sb[:, sc, p, 64 * e:64 * e + 128],
                   es_av[sc][:, sl], start=(sc == 0), stop=(sc == 7))

        def finish_norm(h_av, av, a_sb, fast=False):
            """a = av / D via 1-iteration Newton from the magic seed.
            z1 = (D*y0 - 2)*y0 = -(1/D approx); the final multiply computes
            a = (av * -1) * z1 so no separate sign fix is needed.  The
            (t-2)*y0 step runs on GpSimd (two plain tensor_tensor ops - the
            fused stt opcode is illegal on Pool); fast=True keeps it on DVE
            as one stt for the batch-final head whose chain gates proj."""
            p, e = h_av // 2, h_av % 2
            b0, b1 = 64 * e, 64 * (1 - e)
            y0 = rp.tile([128, T], f32, tag="y0")
            z = rp.tile([128, T], f32, tag="z")
            if fast:
                # per-half pipelined all-DVE chain: half 0's lane-shift DMA
                # overlaps half 1's compute (used for the batch-final heads
                # whose chain latency gates proj)
                for sl in (slice(0, 512), slice(512, T)):
                    nc.vector.tensor_tensor(
                        out=y0[b1:b1 + 64, sl].bitcast(i32),
                        in0=magic_sb[b1:b1 + 64, 0:1].to_broadcast((64, 512)),
                        in1=av[b1:b1 + 64, sl].bitcast(i32), op=ALU.subtract)
                    nc.vector.tensor_tensor(
                        out=z[b1:b1 + 64, sl], in0=av[b1:b1 + 64, sl],
                        in1=y0[b1:b1 + 64, sl], op=ALU.mult)
                    nc.vector.scalar_tensor_tensor(
                        out=z[b1:b1 + 64, sl], in0=z[b1:b1 + 64, sl],
                        scalar=2.0, in1=y0[b1:b1 + 64, sl],
                        op0=ALU.subtract, op1=ALU.mult)
                    nc.sync.dma_start(out=z[b0:b0 + 64, sl], in_=z[b1:b1 + 64, sl])
                    nc.vector.scalar_tensor_tensor(
                        out=a_sb[b0:b0 + 64, p, sl],
                        in0=av[b0:b0 + 64, sl], scalar=-1.0,
                        in1=z[b0:b0 + 64, sl], op0=ALU.mult, op1=ALU.mult)
                return
            nc.vector.tensor_tensor(   # y0 = bits(magic - D_bits)
                out=y0[b1:b1 + 64, :].bitcast(i32),
                in0=magic_sb[b1:b1 + 64, 0:1].to_broadcast((64, T)),
                in1=av[b1:b1 + 64, :].bitcast(i32), op=ALU.subtract)
            nc.vector.tensor_tensor(   # z = D*y0
                out=z[b1:b1 + 64, :], in0=av[b1:b1 + 64, :],
                in1=y0[b1:b1 + 64, :], op=ALU.mult)
            nc.gpsimd.tensor_tensor(
                out=z[b1:b1 + 64, :], in0=z[b1:b1 + 64, :],
                in1=two_sb[b1:b1 + 64, 0:1].to_broadcast((64, T)),
                op=ALU.subtract)
            nc.gpsimd.tensor_tensor(
                out=z[b1:b1 + 64, :], in0=z[b1:b1 + 64, :],
                in1=y0[b1:b1 + 64, :], op=ALU.mult)
            nc.sync.dma_start(out=z[b0:b0 + 64, :], in_=z[b1:b1 + 64, :])
            nc.vector.scalar_tensor_tensor(
                out=a_sb[b0:b0 + 64, p, :],
                in0=av[b0:b0 + 64, :], scalar=-1.0,
                in1=z[b0:b0 + 64, :], op0=ALU.mult, op1=ALU.mult)

        def proj_out(b, a_sb, ms=None):
            x_sb = x_tiles[b]
            for m in (range(KO) if ms is None else ms):
                po = psS.tile([128, T], f32, tag="st")
                for half in range(2):
                    sl = slice(512 * half, 512 * (half + 1))
                    for ko in (1, 2, 3, 0):
                        mm(po[:, sl], wp_sb[:, ko, 128 * m:128 * (m + 1)],
                           a_sb[:, ko, sl], start=(ko == 1), stop=(ko == 0))
                nc.vector.tensor_add(x_sb[:, m, :], po[:], x_sb[:, m, :])
                nc.sync.dma_start(out_d[b, :, m, :], x_sb[:, m, :])

        def proj_out_waves(b, a_sb):
            """Final-batch proj: the ko1-3 accumulations for ALL m run first
            (they only need heads 2..7, long done) so the PE stays warm while
            the tail pair's softmax-normalize chains finish; only the ko0
            wave waits on them.  Uses psB for 2 extra open PSUM groups."""
            x_sb = x_tiles[b]
            pos = []
            for m in range(KO):
                pool = psS if m < 2 else psB
                pos.append(pool.tile([128, T], f32, tag="st" if m < 2 else "av",
                                     name=f"po{m}"))
            for ko in (1, 2, 3):
                for m in range(KO):
                    for half in range(2):
                        sl = slice(512 * half, 512 * (half + 1))
                        mm(pos[m][:, sl], wp_sb[:, ko, 128 * m:128 * (m + 1)],
                           a_sb[:, ko, sl], start=(ko == 1), stop=False)
            for m in range(KO):
                for half in range(2):
                    sl = slice(512 * half, 512 * (half + 1))
                    mm(pos[m][:, sl], wp_sb[:, 0, 128 * m:128 * (m + 1)],
                       a_sb[:, 0, sl], start=False, stop=True)
                nc.vector.tensor_add(x_sb[:, m, :], pos[m][:], x_sb[:, m, :])
                nc.sync.dma_start(out_d[b, :, m, :], x_sb[:, m, :])

        # ---------------- emission ----------------
        # Stage A(0): GroupNorm batch 0
        gn_stats(0)
        h_sbs = [None, None]
        q_sbs = [None, None]
        a_sbs = [None, None]
        h_sbs[0] = hp.tile([128, KO, T], dt_h, tag="h", name="h0")
        gn_normalize(0, gn_bcast(0), h_sbs[0])

        FIRST_MS = (5, 1)            # k/q chunks of head pair 1
        REST_MS = (6, 2, 7, 3, 4, 0)  # pairs 2,3 then pair 0
        HEAD_LOOP = (4, 5, 6, 7, 0, 1)

        def stage_B(b):
            """QKV + vT with St(h0)/St(h1) spliced in right after their
            inputs (k pair 0 = m4, q pair 0 = m0) are available, so ScalarE
            exp starts ~20us before the attention stage.  The vT groups are
            interleaved into St(h0)'s stream and the remaining QKV groups
            into St(h1)'s, so the PE stays dense while exp paces the St
            psum ring."""
            h_sb, q_sb = h_sbs[b], q_sbs[b]
            es01 = {}
            for m in FIRST_MS:
                qkv_group(m, h_sb, q_sb)
            es01[0] = st_block(2, q_sb,
                               av_hook=lambda sc: vt_group(sc, h_sb))

            def qkv_hook(sc):
                if sc < len(REST_MS):
                    qkv_group(REST_MS[sc], h_sb, q_sb)
            es01[1] = st_block(3, q_sb, av_hook=qkv_hook)
            return es01

        def stage_C(b, es01, hooks=None):
            """Attention heads: St(h) interleaved with AV(h-2); finishes."""
            q_sb, a_sb = q_sbs[b], a_sbs[b]
            es_prev2, es_prev1 = es01[0], es01[1]  # pending-AV heads
            hp2, hp1 = 2, 3
            for i, h in enumerate(HEAD_LOOP):
                avp = psB.tile([128, T], f32, tag="av", name="avp")
                hook = (lambda sc: av_mms(avp, hp2, es_prev2, sc))
                es_h = st_block(h, q_sb, av_hook=hook)
                finish_norm(hp2, avp, a_sb, fast=True)
                es_prev2, hp2 = es_prev1, hp1
                es_prev1, hp1 = es_h, h
                if hooks and i in hooks:
                    hooks[i]()
            # batch-final two heads (pair 0): first chain on GpSimd, second
            # all-DVE, so the two run concurrently and proj's last-accumulated
            # chunk (ko0) unblocks sooner.
            for es_t, h_t in ((es_prev2, hp2), (es_prev1, hp1)):
                avp = psB.tile([128, T], f32, tag="av", name="avp")
                for sc in range(8):
                    av_mms(avp, h_t, es_t, sc)
                finish_norm(h_t, avp, a_sb, fast=True)

        q_sbs[0] = qkp.tile([128, KO, T], dt_att, tag="q", name="q0")
        a_sbs[0] = ap_.tile([128, KO, T], dt_a, tag="a", name="a0")
        es01_0 = stage_B(0)

        # batch-1 GN lands in batch-0's attention DVE slack
        def hook_gn1():
            gn_stats(1)

        def hook_norm1():
            h_sbs[1] = hp.tile([128, KO, T], dt_h, tag="h", name="h1")
            gn_normalize(1, gn_bcast(1), h_sbs[1])

        bp_add(0)
        q_sbs[1] = qkp.tile([128, KO, T], dt_att, tag="q", name="q1")
        a_sbs[1] = ap_.tile([128, KO, T], dt_a, tag="a", name="a1")

        def stage_B1_main():
            h_sb, q_sb = h_sbs[1], q_sbs[1]
            es01 = {}
            es01[0] = st_block(2, q_sb,
                               av_hook=lambda sc: vt_group(sc, h_sb))

            def qkv_hook(sc):
                if sc < len(REST_MS):
                    qkv_group(REST_MS[sc], h_sb, q_sb)
            es01[1] = st_block(3, q_sb, av_hook=qkv_hook)
            return es01

        def hook_b1pre():
            for m in FIRST_MS:
                qkv_group(m, h_sbs[1], q_sbs[1])

        stage_C(0, es01_0, hooks={1: hook_gn1, 3: hook_norm1, 4: hook_b1pre})

        # batch-1 QKV/vT fill the PE while batch-0's last softmax-normalize
        # chain completes; then batch-0 proj.
        es01_1 = stage_B1_main()
        bp_add(1)
        stage_C(1, es01_1, hooks={
            1: lambda: proj_out(0, a_sbs[0], ms=[0, 1]),
            3: lambda: proj_out(0, a_sbs[0], ms=[2, 3]),
        })
        proj_out_waves(1, a_sbs[1])

    if not nc.is_finalized():
        nc.finalize()
    return nc


def _prep_inputs(x, norm_w, norm_b, qkv_w, qkv_b, proj_w, proj_b):
    """Fold norms/biases/scale into weights; reshape for the kernel layout."""
    f = np.float32
    x = np.asarray(x, f)
    nw = np.asarray(norm_w, f)
    nb = np.asarray(norm_b, f)
    qkv_w = np.asarray(qkv_w, f)
    qkv_b = np.asarray(qkv_b, f)
    proj_w = np.asarray(proj_w, f)
    proj_b = np.asarray(proj_b, f)

    Wq, Wk, Wv = qkv_w[0:C], qkv_w[C:2 * C], qkv_w[2 * C:3 * C]
    bqv, bkv, bvv = qkv_b[0:C], qkv_b[C:2 * C], qkv_b[2 * C:3 * C]
    scale = f(1.0 / np.sqrt(CH))
    Wq_e = (Wq * nw[None, :]) * scale
    bq_e = (Wq @ nb + bqv) * scale
    Wk_e = Wk * nw[None, :]          # k bias dropped (softmax shift invariance)
    Wv_e = Wv * nw[None, :]
    bv_e = Wv @ nb + bvv
    bp_e = proj_b + proj_w @ bv_e    # v bias folded into proj bias

    def chan_chunks(vec):  # [C] -> [128, KO]
        return np.ascontiguousarray(vec.reshape(KO, 128).T)

    def lhsT_chunks(wT, dtype):  # [C, M] -> [128, KO, M]
        return np.ascontiguousarray(
            wT.reshape(KO, 128, wT.shape[1]).transpose(1, 0, 2)).astype(dtype)

    wqkT = np.concatenate([Wq_e, Wk_e], axis=0).T  # [C, 1024]
    gm = np.zeros((C, NG), f)
    gm[np.arange(C), np.arange(C) // (C // NG)] = 1.0 / (C // NG)
    bm = np.zeros((128, C), f)
    bm[np.arange(C) // (C // NG), np.arange(C)] = 1.0

    dqkv = _npdt(MM_QKV)
    dproj = _npdt(MM_PROJ)
    shared = {
        "wqkT": lhsT_chunks(wqkT, dqkv),
        "wvT": lhsT_chunks(Wv_e.T, dqkv),
        "wpT": lhsT_chunks(proj_w.T, dproj),
        "bq": chan_chunks(bq_e),
        "bp": chan_chunks(bp_e),
        "gmat": np.ascontiguousarray(
            gm.reshape(KO, 128, NG).transpose(1, 0, 2)),
        "bmat": np.ascontiguousarray(bm.reshape(128, KO, 128)),
        "ones": np.ones((128, 64), _npdt(MM_ATT)),
    }
    xr = x.reshape(B, C, T)
    in_maps = []
    for c in range(NCORES):
        xc = xr[c * BPC:(c + 1) * BPC].reshape(BPC, KO, 128, T).transpose(0, 2, 1, 3)
        m = dict(shared)
        m["x"] = np.ascontiguousarray(xc)
        in_maps.append(m)
    return in_maps


def kernel(x, norm_w, norm_b, qkv_w, qkv_b, proj_w, proj_b):
    from concourse.bass_utils import run_bass_kernel_spmd

    in_maps = _prep_inputs(x, norm_w, norm_b, qkv_w, qkv_b, proj_w, proj_b)
    nc = _build_nc()
    res = run_bass_kernel_spmd(nc, in_maps, core_ids=list(range(NCORES)), trace=TRACE)
    kernel.last_results = res
    outs = []
    for c in range(NCORES):
        oc = res.results[c]["out"]  # [BPC, 128, KO, T]
        outs.append(np.asarray(oc).transpose(0, 2, 1, 3).reshape(BPC, C, T))
    full = np.concatenate(outs, axis=0).reshape(B, C, 32, 32).astype(np.float32)
    return full
